# revision 39
# baseline (speedup 1.0000x reference)
"""Trainium2 Bass kernel for nn_BeAttentionGPT (single-head causal attention GPT block).

Computation per batch b (B=8, S=2048, H=1024):
    Q = x @ Wq.T + bq ; K = x @ Wk.T + bk ; V = x @ Wv.T + bv
    scores = Q @ K.T / sqrt(H), causal+pad masked (masked -> -1e9)
    attn = softmax(scores); out = attn @ V
Fully-padded query rows degenerate to a uniform average of all V rows.

Sharding: data-parallel over batch -- one batch per NeuronCore (8 cores).

Algebraic restructuring:
    Q.K^T = x A x^T + u.x_k + v.x_q + bq.bk   with A = Wq^T Wk, u = bq Wk.
    The per-q term and the constant cancel in the kernel's own row
    normalization and are dropped; u.x_k is folded host-side into the exp
    bias.  The device computes Z = A xc^T, S^T = Z contracted with xc^T,
    P = exp(S^T*sscale + bias), V0 = xc Wv^T (bv re-added on host), and
    out = normalized P^T V0 (row sums via a ones-column matmul).

Mask compaction: the pad mask invalidates ~half the positions; q and k
share it, so the host gathers the valid rows of x once into compacted
xc [SC=1152, H] (zero-padded).  Because the gather preserves order,
causality in compacted coordinates is STILL the triangular mask, so the
static causal trapezoid survives compaction.

fp8 acceleration: the Z = A xc^T stage (the largest single GEMM) runs in
fp8 e4m3 with DoubleRow perf mode (K=256 per instruction, 2x bf16
throughput).  A and xc are pre-scaled host-side (x64 / x8) to sit in
e4m3's sweet spot; the combined 512x factor is divided out of the exp
scale.  Everything downstream of the softmax stays bf16 -- quantizing P,
V0, or the V projection would blow the error budget, but score-side
noise is softened by softmax row normalization (measured ~1.5e-2
end-to-end vs the 2e-2 budget).

The scores and attention-out (PV) stages are interleaved chunk-by-chunk
so the final q-block's 9-chunk accumulation does not sit serially at the
kernel tail; its first 8 chunks are issued early and only the diagonal
chunk continuation remains after the last score tile.

All transposes/casts/packing are host-side numpy; DMAs are few and
large, in need-order, on the two HWDGE rings (sync + scalar).
"""

import numpy as np
import ml_dtypes

B, S, H = 8, 2048, 1024
P = 128
SB = 512                 # column-group width
NH = H // P              # 8 h-chunks
SC = 1152                # compacted position capacity (9 chunks of 128)
NSC = SC // P            # 9 compacted chunks
SCALE = 1.0 / float(np.sqrt(np.float32(H)))
ASCALE = 64.0            # host pre-scale of A before e4m3 quantization
XSCALE = 8.0             # host pre-scale of xc before e4m3 quantization
SSCALE = SCALE / (ASCALE * XSCALE)   # exp scale: undoes the fp8 pre-scales
KBIAS = -30000.0         # dummy-key bias: exp(s*sscale - 30000) == 0
CAP = -60000.0 * ASCALE * XSCALE     # causal diag cap in device score units
Q2W = 34                 # valid q-columns in group 2 (max nk - 1024), set at build

_CACHE = {}

# column groups over the 1152 compacted columns
CG = [(0, 512), (512, 512), (1024, 128)]


def _build_program(q2w=Q2W):
    import concourse.bacc as bacc
    import concourse.tile as tile
    from concourse import mybir

    f32 = mybir.dt.float32
    bf16 = mybir.dt.bfloat16
    f8 = mybir.dt.float8e4
    AF = mybir.ActivationFunctionType
    ALU = mybir.AluOpType
    DR = mybir.MatmulPerfMode.DoubleRow

    nc = bacc.Bacc("TRN2", target_bir_lowering=False, debug=False)

    # ---- DRAM I/O ----
    # A^T (pre-scaled, fp8) packed cb-major: host layout [128p, 8cb, 8h, 128c]
    ATp_d = nc.dram_tensor("ATp", [P, NH * H], f8, kind="ExternalInput").ap()
    # xc^T (pre-scaled, fp8).  Group 0 is s-major ([p, 4s, 8h, 128c]) so the
    # first Z chains gate on a single 128KB chunk; groups 1/2 are h-major
    # ([p, 8h, s, c]) so a DoubleRow rhs [p, 2h, s, c] spans 512 columns.
    xq0_d = nc.dram_tensor("xq0", [P, NH * SB], f8, kind="ExternalInput").ap()
    xq1_d = nc.dram_tensor("xq1", [P, NH * SB], f8, kind="ExternalInput").ap()
    xq2_d = nc.dram_tensor("xq2", [P, NH * P], f8, kind="ExternalInput").ap()
    # xc^T bf16 packed chunk-major [128p, 9s, 8h, 128c] (scores rhs + V0 lhsT)
    xcp_d = nc.dram_tensor("xcp", [P, NH * SC], bf16, kind="ExternalInput").ap()
    # Wv^T packed half-major: [128p, 2half, 8h, 512o]
    wvp_d = nc.dram_tensor("wvp", [P, NH * H], bf16, kind="ExternalInput").ap()
    # kbias [P, NSC] ++ tri_cap [P, P], one small f32 transfer
    small_d = nc.dram_tensor("smallf", [P, NSC + P], f32, kind="ExternalInput").ap()
    # output in bf16: halves the output-DMA drain that forms the kernel
    # tail; adds ~0.4% RMS rounding, far inside the error budget.  Only the
    # q2w valid q-rows of the last block are computed/written.
    out_d = nc.dram_tensor("out", [8 * P + q2w, H], bf16,
                           kind="ExternalOutput").ap()

    GOFF = [NH * g0 for g0, gw in CG]  # flat column offset of each group

    with tile.TileContext(nc) as tc:
        from contextlib import ExitStack

        with ExitStack() as ctx:
            consts = ctx.enter_context(tc.tile_pool(name="consts", bufs=1))
            big = ctx.enter_context(tc.tile_pool(name="big", bufs=1))
            pt_pool = ctx.enter_context(tc.tile_pool(name="pt", bufs=1))
            PT_BUFS = {0: 4, 1: 8, 2: 9}  # live P tiles per column group
            out_pool = ctx.enter_context(tc.tile_pool(name="outp", bufs=4))
            small = ctx.enter_context(tc.tile_pool(name="small", bufs=4))
            psT = ctx.enter_context(tc.tile_pool(name="psT", bufs=2, space="PSUM"))
            psA = ctx.enter_context(tc.tile_pool(name="psA", bufs=4, space="PSUM"))

            # ---- constants ----
            ones_col = consts.tile([P, 1], bf16, tag="onesc")
            smallf = consts.tile([P, NSC + P], f32, tag="smallf")
            kbias_sb = smallf[:, 0:NSC]
            tri_sb = smallf[:, NSC:NSC + P]
            # PE warmup fodder: an all-zero tile, matmul'd while the first
            # input DMAs are in flight so the PE's DVFS ramp completes before
            # real work arrives.  The warmup psum (all zeros) is consumed by
            # the ones_col producer: ones = (0 * 0) + 1.
            wm = consts.tile([P, SB], bf16, tag="warm")
            nc.vector.memset(wm, 0.0)

            at8 = big.tile([P, NH * H], f8, tag="at8")       # [p, 8cb*8h*128c]
            xq = [
                big.tile([P, NH * SB], f8, tag="xq0", name="xq0_t"),
                big.tile([P, NH * SB], f8, tag="xq1", name="xq1_t"),
                big.tile([P, NH * P], f8, tag="xq2", name="xq2_t"),
            ]
            xc = big.tile([P, NH * SC], bf16, tag="xc")      # [p, 9s*8h*128c]
            wv = big.tile([P, NH * H], bf16, tag="wv")       # [p, 2half*8h*512o]
            z = big.tile([P, NH * SC], bf16, tag="z")        # [p, 3g(8h*gw)]
            v = big.tile([P, NSC * H], bf16, tag="v")        # [p, 9s*1024o]

            at4 = at8.rearrange("p (cb h c) -> p cb h c", cb=NH, h=NH)
            xq4 = [
                xq[0].rearrange("p (h s c) -> p h s c", h=NH, s=SB // P),
                xq[1].rearrange("p (h s c) -> p h s c", h=NH, s=SB // P),
                xq[2].rearrange("p (h s c) -> p h s c", h=NH, s=1),
            ]
            xc4 = xc.rearrange("p (s h c) -> p s h c", s=NSC, h=NH)

            def xc_rhs(gi, h, qoff):  # group gi columns qoff.. as 3D AP
                g0, gw = CG[gi]
                return xc4[:, (g0 + qoff) // P:(g0 + gw) // P, h, :]

            def xc_chunk(s, h):
                return xc4[:, s, h, :]

            def z_sl(gi, h, c0, c1):
                g0, gw = CG[gi]
                base = GOFF[gi] + h * gw
                return z[:, base + c0:base + c1]

            def wv_sl(h, half):  # half-major pack
                base = half * (NH * SB) + h * SB
                return wv[:, base:base + SB]

            # chunk s (0..8) -> (group, column offset within group)
            def chunk_loc(s):
                gi = 0 if s < 4 else (1 if s < 8 else 2)
                return gi, s * P - CG[gi][0]

            # ---- input DMA ----
            # Need-order, on the two HWDGE rings (sync, scalar).  A single
            # InstDMACopy spreads its descriptors over all 16 SDMA engines,
            # so one transfer already approaches the HBM ceiling; the
            # startup-critical pieces (A^T block 0, xq group-0 chunk 0) are
            # split small because the consumer waits on the completion
            # semaphore of the WHOLE transfer (+~2us HBM write receipt).
            def dcols(eng, dst, src, c0, c1):
                eng.dma_start(out=dst[:, c0:c1], in_=src[:, c0:c1])

            # The ~2.4us HBM-write-receipt latency on every completion
            # semaphore dominates the gate, so fine splitting buys nothing:
            # ship each tensor whole, in need-order, and let the PE warmup
            # chain absorb the wait.
            # cb1 rides the scalar ring right behind xq0 so the hp1 chain's
            # gate (transfer + ~2.4us receipt) lands before hp0 finishes;
            # cb2-3 / cb4-7 receipts on sync stay ahead of their consumers.
            dcols(nc.sync, at8, ATp_d, 0, H)            # cb0 (gate, 128KB)
            dcols(nc.scalar, xq[0], xq0_d, 0, NH * SB)  # Z group-0 rhs (gate)
            dcols(nc.sync, at8, ATp_d, 2 * H, 4 * H)    # cb2-3
            dcols(nc.scalar, at8, ATp_d, H, 2 * H)      # cb1 (128KB)
            dcols(nc.sync, at8, ATp_d, 4 * H, NH * H)   # cb4-7
            dcols(nc.scalar, xq[1], xq1_d, 0, NH * SB)
            dcols(nc.scalar, xq[2], xq2_d, 0, NH * P)
            nc.sync.dma_start(out=smallf, in_=small_d)
            dcols(nc.sync, wv, wvp_d, 0, NH * SB)
            h2 = NH * SC // 2
            dcols(nc.scalar, xc, xcp_d, 0, h2)
            dcols(nc.sync, wv, wvp_d, NH * SB, NH * H)
            dcols(nc.scalar, xc, xcp_d, h2, NH * SC)

            # ---- PE warmup + ones_col production ----
            warm_ps = psA.tile([P, SB], f32, tag="psA", name="warm_ps")
            NWARM = 21
            for k in range(NWARM):
                nc.tensor.matmul(
                    warm_ps, lhsT=wm[:, 0:P], rhs=wm,
                    start=(k == 0), stop=(k == NWARM - 1),
                )
            nc.vector.tensor_scalar(
                ones_col, warm_ps[:, 0:1], 0.0, 1.0,
                ALU.mult, ALU.add,
            )

            evict_ctr = [0]

            def evict(dst, src):
                if evict_ctr[0] % 2 == 0:
                    nc.scalar.activation(dst, src, AF.Copy)
                else:
                    nc.vector.tensor_copy(dst, src)
                evict_ctr[0] += 1

            # ---- Z = A xc^T in fp8 DoubleRow (sequential accumulation) ----
            for gi in (0, 1, 2):
                g0, gw = CG[gi]
                for hp in range(NH):
                    ps = psA.tile([P, gw], f32, tag="psA", name="psA_t")
                    for h in range(0, NH, 2):
                        nc.tensor.matmul(
                            ps,
                            lhsT=at4[:, hp, h:h + 2, :],
                            rhs=xq4[gi][:, h:h + 2, :, :],
                            start=(h == 0),
                            stop=(h == NH - 2),
                            perf_mode=DR,
                        )
                    evict(z_sl(gi, hp, 0, gw), ps)

            # ---- V0 projection (no bias; bv re-added host-side) ----
            for half in range(2):
                for s in range(NSC):
                    ps = psA.tile([P, SB], f32, tag="psA", name="psA_t")
                    for h in range(NH):
                        nc.tensor.matmul(
                            ps,
                            lhsT=xc_chunk(s, h),
                            rhs=wv_sl(h, half),
                            start=(h == 0),
                            stop=(h == NH - 1),
                        )
                    evict(v[:, s * H + half * SB:s * H + (half + 1) * SB], ps)

            # ---- scores + attention-out, interleaved by k-chunk ----
            # scores: S^T[k~, q~] = sum_h Z[h, k~] xc[q~, h]; causal trapezoid
            # in compacted coords (skip strictly-upper tiles, min-cap the
            # diagonal 128x128 with the tril constant).  After chunk i's score
            # tiles are issued, q-block i-1's PV chains are issued -- its exp
            # runs on the scalar engine under chunk i's PE work, so the PE
            # never waits and the kernel tail holds only the last block's
            # diagonal-chunk continuation.
            pts = {}

            def scores_chunk(i):
                ig, ioff = chunk_loc(i)
                for gi, (g0, gw) in enumerate(CG):
                    gwe = q2w if gi == 2 else gw  # only q2w valid q in group 2
                    if i * P >= g0 + gwe:
                        continue  # strictly above the causal diagonal
                    qoff = max(i * P - g0, 0)
                    ps = psA.tile([P, gwe], f32, tag="psA", name="psA_t")
                    for h in range(NH):
                        rhs = (xc4[:, NSC - 1, h, 0:q2w] if gi == 2
                               else xc_rhs(gi, h, qoff))
                        nc.tensor.matmul(
                            ps[:, qoff:gwe],
                            lhsT=z_sl(ig, h, ioff, ioff + P),
                            rhs=rhs,
                            start=(h == 0),
                            stop=(h == NH - 1),
                        )
                    if i * P >= g0:  # diagonal-crossing tile
                        w = min(P, gwe - qoff)
                        nc.vector.tensor_tensor(
                            ps[:, qoff:qoff + w],
                            ps[:, qoff:qoff + w],
                            tri_sb[:, 0:w],
                            ALU.min,
                        )
                    pt = pt_pool.tile([P, gwe], bf16, tag=f"pt{gi}",
                                      bufs=PT_BUFS[gi], name="pt_t")
                    nc.scalar.activation(
                        pt[:, qoff:gwe], ps[:, qoff:gwe], AF.Exp,
                        bias=kbias_sb[:, i:i + 1],
                        scale=SSCALE,
                    )
                    pts[(i, gi)] = pt

            blk_state = {}  # j -> (ops, sps) for split-chain continuation

            def blk_geom(j):
                gi, (g0, gw) = next(
                    (g, cg) for g, cg in enumerate(CG)
                    if cg[0] <= j * P < cg[0] + cg[1]
                )
                qw = q2w if gi == 2 else P  # valid q rows in this block
                return gi, j * P - g0, qw

            def pv_chains(j, i_lo, i_hi, close):
                """q-block j attention-out matmuls, chunks i_lo..i_hi."""
                gi, qo, qw = blk_geom(j)
                if j in blk_state:
                    ops, sps = blk_state[j]
                else:
                    ops = psT.tile([P, H], f32, tag="psT", name="psO_t")
                    sps = psA.tile([P, 1], f32, tag="psA", name="psS_t")
                    blk_state[j] = (ops, sps)
                for half in range(2):
                    for i in range(i_lo, i_hi + 1):
                        nc.tensor.matmul(
                            ops[0:qw, half * SB:(half + 1) * SB],
                            lhsT=pts[(i, gi)][:, qo:qo + qw],
                            rhs=v[:, i * H + half * SB:i * H + (half + 1) * SB],
                            start=(i == 0),
                            stop=(close and i == i_hi),
                            skip_group_check=True,
                        )
                for i in range(i_lo, i_hi + 1):
                    nc.tensor.matmul(
                        sps[0:qw, :], lhsT=pts[(i, gi)][:, qo:qo + qw],
                        rhs=ones_col,
                        start=(i == 0), stop=(close and i == i_hi),
                        skip_group_check=True,
                    )
                return blk_state[j]

            def pv_finish(j, halves_split):
                """reciprocal + eviction + output DMA for a closed block."""
                _, _, qw = blk_geom(j)
                ops, sps = blk_state[j]
                rr = small.tile([P, 1], f32, tag="rr", name="rr_t")
                nc.vector.reciprocal(rr[0:qw, :], sps[0:qw, :])
                pieces = 2 if halves_split else 2
                pw = H // pieces
                for pc in range(pieces):
                    outsb = out_pool.tile([P, pw], bf16, tag="outp",
                                          bufs=4, name="outsb_t")
                    if (j + pc) % 2 == 0:
                        nc.scalar.activation(
                            outsb[0:qw, :], ops[0:qw, pc * pw:(pc + 1) * pw],
                            AF.Copy, scale=rr,
                        )
                    else:
                        nc.vector.tensor_scalar_mul(
                            outsb[0:qw, :], ops[0:qw, pc * pw:(pc + 1) * pw],
                            rr,
                        )
                    eng = nc.sync if (j + pc) % 2 == 0 else nc.scalar
                    eng.dma_start(
                        out=out_d[j * P:j * P + qw, pc * pw:(pc + 1) * pw],
                        in_=outsb[0:qw, :],
                    )

            def pv_block(j, i_lo, i_hi, final):
                pv_chains(j, i_lo, i_hi, close=final)
                if final:
                    pv_finish(j, halves_split=False)

            last = NSC - 1
            for i in range(NSC):
                scores_chunk(i)
                if i >= 2:
                    pv_block(i - 2, 0, i - 2, final=True)
            # block 7 full (its eviction overlaps block 8's chains), then
            # block 8 minus its diagonal chunk, then the sums-first
            # continuation so only ~1 matmul + 2 evict/DMA pairs trail.
            pv_block(last - 1, 0, last - 1, final=True)
            pv_block(last, 0, last - 1, final=False)
            gi8, qo8, qw8 = blk_geom(last)
            ops8, sps8 = blk_state[last]
            nc.tensor.matmul(
                sps8[0:qw8, :], lhsT=pts[(last, gi8)][:, qo8:qo8 + qw8],
                rhs=ones_col, start=False, stop=True, skip_group_check=True,
            )
            rr8 = small.tile([P, 1], f32, tag="rr", name="rr_t")
            nc.vector.reciprocal(rr8[0:qw8, :], sps8[0:qw8, :])
            # both half continuations BEFORE any eviction: the tile-granular
            # psum hazard tracking otherwise serializes the second half's
            # matmul behind the first half's eviction.
            for half in range(2):
                nc.tensor.matmul(
                    ops8[0:qw8, half * SB:(half + 1) * SB],
                    lhsT=pts[(last, gi8)][:, qo8:qo8 + qw8],
                    rhs=v[:, last * H + half * SB:last * H + (half + 1) * SB],
                    start=False, stop=True, skip_group_check=True,
                )
            # evict halves on both engines in parallel, then one DMA: a single
            # transfer puts one (final) HBM write receipt on the critical
            # path instead of two.
            outsb8 = out_pool.tile([P, H], bf16, tag="outw",
                                   bufs=1, name="outsb8_t")
            nc.scalar.activation(
                outsb8[0:qw8, 0:SB], ops8[0:qw8, 0:SB],
                AF.Copy, scale=rr8[0:qw8, :],
            )
            nc.vector.tensor_scalar_mul(
                outsb8[0:qw8, SB:H], ops8[0:qw8, SB:H], rr8[0:qw8, :],
            )
            nc.sync.dma_start(
                out=out_d[last * P:last * P + qw8, :],
                in_=outsb8[0:qw8, :],
            )

    nc.compile()
    return nc


def _get_program(q2w=Q2W):
    key = ("nc", q2w)
    if key not in _CACHE:
        _CACHE[key] = _build_program(q2w)
    return _CACHE[key]


def _host_reference(xb, mb, Wq, bq, Wk, bk, Wv, bv):
    """Exact (f64) per-batch fallback, mirrors the reference computation."""
    xb = xb.astype(np.float64)
    Q = xb @ Wq.astype(np.float64).T + bq.astype(np.float64)
    K = xb @ Wk.astype(np.float64).T + bk.astype(np.float64)
    V = xb @ Wv.astype(np.float64).T + bv.astype(np.float64)
    sc = Q @ K.T / np.sqrt(np.float64(H))
    keep = np.tril(np.ones((S, S), bool)) & (mb[None, :] & mb[:, None])
    sc = np.where(keep, sc, -1e9)
    sc -= sc.max(axis=1, keepdims=True)
    Pm = np.exp(sc)
    return ((Pm @ V) / Pm.sum(axis=1, keepdims=True)).astype(np.float32)


def _make_in_maps(x, attention_mask, Wq, bq, Wk, bk, Wv, bv):
    bf16 = ml_dtypes.bfloat16
    f8 = ml_dtypes.float8_e4m3
    f32 = np.float32
    in_maps = []
    fallback = []
    valid_idx = []
    # A = Wq^T Wk  =>  A^T = Wk^T Wq; pack [128p, 8cb, 8h, 128c], fp8 scaled
    AT = (Wk.astype(f32).T @ Wq.astype(f32)) * f32(ASCALE)
    ATp = np.ascontiguousarray(
        AT.reshape(NH, P, NH, P).transpose(1, 2, 0, 3).reshape(P, NH * H)
    ).astype(f8)
    # Wv^T pack half-major [128p, 2half, 8h, 512o]
    WvT = Wv.astype(f32).T.astype(bf16)
    wvp = np.ascontiguousarray(
        WvT.reshape(NH, P, 2, SB).transpose(1, 2, 0, 3).reshape(P, NH * H))
    u = bq.astype(f32) @ Wk.astype(f32)  # [H]; per-k score bias u.x_k
    ii = np.arange(P)
    tri_cap = np.where(
        ii[:, None] > ii[None, :], f32(CAP), f32(3.0e38)
    ).astype(f32)
    for b in range(B):
        mb = attention_mask[b].astype(bool)
        xb = x[b].astype(f32)
        idx = np.nonzero(mb)[0]
        if len(idx) > SC:
            fallback.append(b)
            idx = idx[:SC]
        nk = len(idx)
        valid_idx.append(idx)
        xc = np.zeros((SC, H), dtype=f32)
        xc[:nk] = xb[idx]
        kb = np.full(SC, KBIAS, dtype=f32)
        kb[:nk] = (xc[:nk] @ u) * f32(SCALE)
        smallf = np.concatenate(
            [kb.reshape(NSC, P).T.astype(f32), tri_cap], axis=1)
        # xc^T bf16 packed chunk-major [128p, 9s, 8h, 128c]
        xcT = xc.T.astype(bf16)
        xcp = np.ascontiguousarray(
            xcT.reshape(NH, P, NSC, P).transpose(1, 2, 0, 3).reshape(P, NSC * H))
        # xc^T fp8 (scaled) packed per group, h-major: [128p, 8h, s_in_g, 128c]
        xq8 = (xc.T * f32(XSCALE)).astype(f8)  # [H, SC]
        xq4 = xq8.reshape(NH, P, NSC, P).transpose(1, 0, 2, 3)  # [p, h, s, c]
        def gpack(s0, s1):
            return np.ascontiguousarray(
                xq4[:, :, s0:s1, :].reshape(P, NH * (s1 - s0) * P))
        in_maps.append({
            "ATp": ATp, "wvp": wvp,
            "xcp": np.ascontiguousarray(xcp),
            "xq0": gpack(0, 4), "xq1": gpack(4, 8), "xq2": gpack(8, 9),
            "smallf": np.ascontiguousarray(smallf),
        })
    return in_maps, fallback, valid_idx


def run_spmd(x, attention_mask, Wq, bq, Wk, bk, Wv, bv, **spmd_kwargs):
    """Build (cached), run on 8 cores, return (stacked output, BassKernelResults)."""
    from concourse import bass_utils

    counts = [int(attention_mask[b].astype(bool).sum()) for b in range(B)]
    q2w = min(P, max(1, max(counts) - 8 * P))
    nc = _get_program(q2w)
    in_maps, fallback, valid_idx = _make_in_maps(
        x, attention_mask, Wq, bq, Wk, bk, Wv, bv)
    res = bass_utils.run_bass_kernel_spmd(
        nc, in_maps, core_ids=list(range(B)), **spmd_kwargs
    )
    bvf = bv.astype(np.float32)
    out = np.empty((B, S, H), dtype=np.float32)
    for b in range(B):
        dev = np.asarray(res.results[b]["out"], dtype=np.float32)
        idx = valid_idx[b]
        # scatter compacted rows back; bv was dropped from the device V
        # projection and attn rows sum to 1, so += bv here is exact.
        out[b][idx] = dev[:len(idx)] + bvf
        inv = ~attention_mask[b].astype(bool)
        if inv.any():
            # fully-padded query rows reduce to the uniform mean of all V
            # rows; mean(V) == mean(x) @ Wv.T + bv by linearity.
            mv = (x[b].astype(np.float64).mean(axis=0) @
                  Wv.astype(np.float64).T + bv.astype(np.float64))
            out[b][inv] = mv.astype(np.float32)
    for b in fallback:  # mask had > SC valid keys (never with ~50% masks)
        out[b] = _host_reference(x[b], attention_mask[b].astype(bool),
                                 Wq, bq, Wk, bk, Wv, bv)
    return out, res


def kernel(x, attention_mask, Wq, bq, Wk, bk, Wv, bv):
    x = np.asarray(x)
    attention_mask = np.asarray(attention_mask)
    Wq, bq = np.asarray(Wq), np.asarray(bq)
    Wk, bk = np.asarray(Wk), np.asarray(bk)
    Wv, bv = np.asarray(Wv), np.asarray(bv)
    out, _ = run_spmd(x, attention_mask, Wq, bq, Wk, bk, Wv, bv)
    return out


# revision 41
# speedup vs baseline: 1.0065x; 1.0065x over previous
"""Trainium2 Bass kernel for nn_BeAttentionGPT (single-head causal attention GPT block).

Computation per batch b (B=8, S=2048, H=1024):
    Q = x @ Wq.T + bq ; K = x @ Wk.T + bk ; V = x @ Wv.T + bv
    scores = Q @ K.T / sqrt(H), causal+pad masked (masked -> -1e9)
    attn = softmax(scores); out = attn @ V
Fully-padded query rows degenerate to a uniform average of all V rows.

Sharding: data-parallel over batch -- one batch per NeuronCore (8 cores).

Algebraic restructuring:
    Q.K^T = x A x^T + u.x_k + v.x_q + bq.bk   with A = Wq^T Wk, u = bq Wk.
    The per-q term and the constant cancel in the kernel's own row
    normalization and are dropped; u.x_k is folded host-side into the exp
    bias.  The device computes Z = A xc^T, S^T = Z contracted with xc^T,
    P = exp(S^T*sscale + bias), V0 = xc Wv^T (bv re-added on host), and
    out = normalized P^T V0 (row sums via a ones-column matmul).

Mask compaction: the pad mask invalidates ~half the positions; q and k
share it, so the host gathers the valid rows of x once into compacted
xc [SC=1152, H] (zero-padded).  Because the gather preserves order,
causality in compacted coordinates is STILL the triangular mask, so the
static causal trapezoid survives compaction.

fp8 acceleration: the Z = A xc^T stage (the largest single GEMM) runs in
fp8 e4m3 with DoubleRow perf mode (K=256 per instruction, 2x bf16
throughput).  A and xc are pre-scaled host-side (x64 / x8) to sit in
e4m3's sweet spot; the combined 512x factor is divided out of the exp
scale.  Everything downstream of the softmax stays bf16 -- quantizing P,
V0, or the V projection would blow the error budget, but score-side
noise is softened by softmax row normalization (measured ~1.5e-2
end-to-end vs the 2e-2 budget).

The scores and attention-out (PV) stages are interleaved chunk-by-chunk
so the final q-block's 9-chunk accumulation does not sit serially at the
kernel tail; its first 8 chunks are issued early and only the diagonal
chunk continuation remains after the last score tile.

All transposes/casts/packing are host-side numpy; DMAs are few and
large, in need-order, on the two HWDGE rings (sync + scalar).
"""

import numpy as np
import ml_dtypes

B, S, H = 8, 2048, 1024
P = 128
SB = 512                 # column-group width
NH = H // P              # 8 h-chunks
SC = 1152                # compacted position capacity (9 chunks of 128)
NSC = SC // P            # 9 compacted chunks
SCALE = 1.0 / float(np.sqrt(np.float32(H)))
ASCALE = 64.0            # host pre-scale of A before e4m3 quantization
XSCALE = 8.0             # host pre-scale of xc before e4m3 quantization
SSCALE = SCALE / (ASCALE * XSCALE)   # exp scale: undoes the fp8 pre-scales
KBIAS = -30000.0         # dummy-key bias: exp(s*sscale - 30000) == 0
CAP = -60000.0 * ASCALE * XSCALE     # causal diag cap in device score units
Q2W = 34                 # valid q-columns in group 2 (max nk - 1024), set at build

_CACHE = {}

# column groups over the 1152 compacted columns
CG = [(0, 512), (512, 512), (1024, 128)]


def _build_program(q2w=Q2W):
    import concourse.bacc as bacc
    import concourse.tile as tile
    from concourse import mybir

    f32 = mybir.dt.float32
    bf16 = mybir.dt.bfloat16
    f8 = mybir.dt.float8e4
    AF = mybir.ActivationFunctionType
    ALU = mybir.AluOpType
    DR = mybir.MatmulPerfMode.DoubleRow

    nc = bacc.Bacc("TRN2", target_bir_lowering=False, debug=False)

    # ---- DRAM I/O ----
    # A^T (pre-scaled, fp8) packed cb-major: host layout [128p, 8cb, 8h, 128c]
    ATp_d = nc.dram_tensor("ATp", [P, NH * H], f8, kind="ExternalInput").ap()
    # xc^T (pre-scaled, fp8).  Group 0 is s-major ([p, 4s, 8h, 128c]) so the
    # first Z chains gate on a single 128KB chunk; groups 1/2 are h-major
    # ([p, 8h, s, c]) so a DoubleRow rhs [p, 2h, s, c] spans 512 columns.
    xq0_d = nc.dram_tensor("xq0", [P, NH * SB], f8, kind="ExternalInput").ap()
    xq1_d = nc.dram_tensor("xq1", [P, NH * SB], f8, kind="ExternalInput").ap()
    xq2_d = nc.dram_tensor("xq2", [P, NH * P], f8, kind="ExternalInput").ap()
    # xc^T bf16 packed chunk-major [128p, 9s, 8h, 128c] (scores rhs + V0 lhsT)
    xcp_d = nc.dram_tensor("xcp", [P, NH * SC], bf16, kind="ExternalInput").ap()
    # Wv^T packed half-major: [128p, 2half, 8h, 512o]
    wvp_d = nc.dram_tensor("wvp", [P, NH * H], bf16, kind="ExternalInput").ap()
    # kbias [P, NSC] ++ tri_cap [P, P], one small f32 transfer
    small_d = nc.dram_tensor("smallf", [P, NSC + P], f32, kind="ExternalInput").ap()
    # output in bf16: halves the output-DMA drain that forms the kernel
    # tail; adds ~0.4% RMS rounding, far inside the error budget.  Only the
    # q2w valid q-rows of the last block are computed/written.
    out_d = nc.dram_tensor("out", [8 * P + q2w, H], bf16,
                           kind="ExternalOutput").ap()

    GOFF = [NH * g0 for g0, gw in CG]  # flat column offset of each group

    with tile.TileContext(nc) as tc:
        from contextlib import ExitStack

        with ExitStack() as ctx:
            consts = ctx.enter_context(tc.tile_pool(name="consts", bufs=1))
            big = ctx.enter_context(tc.tile_pool(name="big", bufs=1))
            pt_pool = ctx.enter_context(tc.tile_pool(name="pt", bufs=1))
            PT_BUFS = {0: 4, 1: 8, 2: 9}  # live P tiles per column group
            out_pool = ctx.enter_context(tc.tile_pool(name="outp", bufs=4))
            small = ctx.enter_context(tc.tile_pool(name="small", bufs=4))
            psT = ctx.enter_context(tc.tile_pool(name="psT", bufs=2, space="PSUM"))
            psA = ctx.enter_context(tc.tile_pool(name="psA", bufs=4, space="PSUM"))

            # ---- constants ----
            ones_col = consts.tile([P, 1], bf16, tag="onesc")
            smallf = consts.tile([P, NSC + P], f32, tag="smallf")
            kbias_sb = smallf[:, 0:NSC]
            tri_sb = smallf[:, NSC:NSC + P]
            # PE warmup fodder: an all-zero tile, matmul'd while the first
            # input DMAs are in flight so the PE's DVFS ramp completes before
            # real work arrives.  The warmup psum (all zeros) is consumed by
            # the ones_col producer: ones = (0 * 0) + 1.
            wm = consts.tile([P, SB], bf16, tag="warm")
            nc.vector.memset(wm, 0.0)

            at8 = big.tile([P, NH * H], f8, tag="at8")       # [p, 8cb*8h*128c]
            xq = [
                big.tile([P, NH * SB], f8, tag="xq0", name="xq0_t"),
                big.tile([P, NH * SB], f8, tag="xq1", name="xq1_t"),
                big.tile([P, NH * P], f8, tag="xq2", name="xq2_t"),
            ]
            xc = big.tile([P, NH * SC], bf16, tag="xc")      # [p, 9s*8h*128c]
            wv = big.tile([P, NH * H], bf16, tag="wv")       # [p, 2half*8h*512o]
            z = big.tile([P, NH * SC], bf16, tag="z")        # [p, 3g(8h*gw)]
            v = big.tile([P, NSC * H], bf16, tag="v")        # [p, 9s*1024o]

            at4 = at8.rearrange("p (cb h c) -> p cb h c", cb=NH, h=NH)
            xq4 = [
                xq[0].rearrange("p (h s c) -> p h s c", h=NH, s=SB // P),
                xq[1].rearrange("p (h s c) -> p h s c", h=NH, s=SB // P),
                xq[2].rearrange("p (h s c) -> p h s c", h=NH, s=1),
            ]
            xc4 = xc.rearrange("p (s h c) -> p s h c", s=NSC, h=NH)

            def xc_rhs(gi, h, qoff):  # group gi columns qoff.. as 3D AP
                g0, gw = CG[gi]
                return xc4[:, (g0 + qoff) // P:(g0 + gw) // P, h, :]

            def xc_chunk(s, h):
                return xc4[:, s, h, :]

            def z_sl(gi, h, c0, c1):
                g0, gw = CG[gi]
                base = GOFF[gi] + h * gw
                return z[:, base + c0:base + c1]

            def wv_sl(h, half):  # half-major pack
                base = half * (NH * SB) + h * SB
                return wv[:, base:base + SB]

            # chunk s (0..8) -> (group, column offset within group)
            def chunk_loc(s):
                gi = 0 if s < 4 else (1 if s < 8 else 2)
                return gi, s * P - CG[gi][0]

            # ---- input DMA ----
            # Need-order, on the two HWDGE rings (sync, scalar).  A single
            # InstDMACopy spreads its descriptors over all 16 SDMA engines,
            # so one transfer already approaches the HBM ceiling; the
            # startup-critical pieces (A^T block 0, xq group-0 chunk 0) are
            # split small because the consumer waits on the completion
            # semaphore of the WHOLE transfer (+~2us HBM write receipt).
            def dcols(eng, dst, src, c0, c1):
                eng.dma_start(out=dst[:, c0:c1], in_=src[:, c0:c1])

            # The ~2.4us HBM-write-receipt latency on every completion
            # semaphore dominates the gate, so fine splitting buys nothing:
            # ship each tensor whole, in need-order, and let the PE warmup
            # chain absorb the wait.
            dcols(nc.sync, at8, ATp_d, 0, H)            # cb0 (gate, 128KB)
            dcols(nc.scalar, xq[0], xq0_d, 0, NH * SB)  # Z group-0 rhs (gate)
            dcols(nc.sync, at8, ATp_d, H, 4 * H)        # cb1-3
            dcols(nc.sync, at8, ATp_d, 4 * H, NH * H)   # cb4-7
            dcols(nc.scalar, xq[1], xq1_d, 0, NH * SB)
            dcols(nc.scalar, xq[2], xq2_d, 0, NH * P)
            nc.sync.dma_start(out=smallf, in_=small_d)
            dcols(nc.sync, wv, wvp_d, 0, NH * SB)
            h2 = NH * SC // 2
            dcols(nc.scalar, xc, xcp_d, 0, h2)
            dcols(nc.sync, wv, wvp_d, NH * SB, NH * H)
            dcols(nc.scalar, xc, xcp_d, h2, NH * SC)

            # ---- PE warmup + ones_col production ----
            warm_ps = psA.tile([P, SB], f32, tag="psA", name="warm_ps")
            NWARM = 14
            for k in range(NWARM):
                nc.tensor.matmul(
                    warm_ps, lhsT=wm[:, 0:P], rhs=wm,
                    start=(k == 0), stop=(k == NWARM - 1),
                )
            nc.vector.tensor_scalar(
                ones_col, warm_ps[:, 0:1], 0.0, 1.0,
                ALU.mult, ALU.add,
            )

            evict_ctr = [0]

            def evict(dst, src):
                if evict_ctr[0] % 2 == 0:
                    nc.scalar.activation(dst, src, AF.Copy)
                else:
                    nc.vector.tensor_copy(dst, src)
                evict_ctr[0] += 1

            # ---- Z = A xc^T in fp8 DoubleRow (sequential accumulation) ----
            for gi in (0, 1, 2):
                g0, gw = CG[gi]
                for hp in range(NH):
                    ps = psA.tile([P, gw], f32, tag="psA", name="psA_t")
                    for h in range(0, NH, 2):
                        nc.tensor.matmul(
                            ps,
                            lhsT=at4[:, hp, h:h + 2, :],
                            rhs=xq4[gi][:, h:h + 2, :, :],
                            start=(h == 0),
                            stop=(h == NH - 2),
                            perf_mode=DR,
                        )
                    evict(z_sl(gi, hp, 0, gw), ps)

            # ---- V0 projection (no bias; bv re-added host-side) ----
            for half in range(2):
                for s in range(NSC):
                    ps = psA.tile([P, SB], f32, tag="psA", name="psA_t")
                    for h in range(NH):
                        nc.tensor.matmul(
                            ps,
                            lhsT=xc_chunk(s, h),
                            rhs=wv_sl(h, half),
                            start=(h == 0),
                            stop=(h == NH - 1),
                        )
                    evict(v[:, s * H + half * SB:s * H + (half + 1) * SB], ps)

            # ---- scores + attention-out, interleaved by k-chunk ----
            # scores: S^T[k~, q~] = sum_h Z[h, k~] xc[q~, h]; causal trapezoid
            # in compacted coords (skip strictly-upper tiles, min-cap the
            # diagonal 128x128 with the tril constant).  After chunk i's score
            # tiles are issued, q-block i-1's PV chains are issued -- its exp
            # runs on the scalar engine under chunk i's PE work, so the PE
            # never waits and the kernel tail holds only the last block's
            # diagonal-chunk continuation.
            pts = {}

            def scores_chunk(i):
                ig, ioff = chunk_loc(i)
                for gi, (g0, gw) in enumerate(CG):
                    gwe = q2w if gi == 2 else gw  # only q2w valid q in group 2
                    if i * P >= g0 + gwe:
                        continue  # strictly above the causal diagonal
                    qoff = max(i * P - g0, 0)
                    ps = psA.tile([P, gwe], f32, tag="psA", name="psA_t")
                    for h in range(NH):
                        rhs = (xc4[:, NSC - 1, h, 0:q2w] if gi == 2
                               else xc_rhs(gi, h, qoff))
                        nc.tensor.matmul(
                            ps[:, qoff:gwe],
                            lhsT=z_sl(ig, h, ioff, ioff + P),
                            rhs=rhs,
                            start=(h == 0),
                            stop=(h == NH - 1),
                        )
                    if i * P >= g0:  # diagonal-crossing tile
                        w = min(P, gwe - qoff)
                        nc.vector.tensor_tensor(
                            ps[:, qoff:qoff + w],
                            ps[:, qoff:qoff + w],
                            tri_sb[:, 0:w],
                            ALU.min,
                        )
                    pt = pt_pool.tile([P, gwe], bf16, tag=f"pt{gi}",
                                      bufs=PT_BUFS[gi], name="pt_t")
                    nc.scalar.activation(
                        pt[:, qoff:gwe], ps[:, qoff:gwe], AF.Exp,
                        bias=kbias_sb[:, i:i + 1],
                        scale=SSCALE,
                    )
                    pts[(i, gi)] = pt

            blk_state = {}  # j -> (ops, sps) for split-chain continuation

            def blk_geom(j):
                gi, (g0, gw) = next(
                    (g, cg) for g, cg in enumerate(CG)
                    if cg[0] <= j * P < cg[0] + cg[1]
                )
                qw = q2w if gi == 2 else P  # valid q rows in this block
                return gi, j * P - g0, qw

            def pv_chains(j, i_lo, i_hi, close):
                """q-block j attention-out matmuls, chunks i_lo..i_hi."""
                gi, qo, qw = blk_geom(j)
                if j in blk_state:
                    ops, sps = blk_state[j]
                else:
                    ops = psT.tile([P, H], f32, tag="psT", name="psO_t")
                    sps = psA.tile([P, 1], f32, tag="psA", name="psS_t")
                    blk_state[j] = (ops, sps)
                for half in range(2):
                    for i in range(i_lo, i_hi + 1):
                        nc.tensor.matmul(
                            ops[0:qw, half * SB:(half + 1) * SB],
                            lhsT=pts[(i, gi)][:, qo:qo + qw],
                            rhs=v[:, i * H + half * SB:i * H + (half + 1) * SB],
                            start=(i == 0),
                            stop=(close and i == i_hi),
                            skip_group_check=True,
                        )
                for i in range(i_lo, i_hi + 1):
                    nc.tensor.matmul(
                        sps[0:qw, :], lhsT=pts[(i, gi)][:, qo:qo + qw],
                        rhs=ones_col,
                        start=(i == 0), stop=(close and i == i_hi),
                        skip_group_check=True,
                    )
                return blk_state[j]

            def pv_finish(j, halves_split):
                """reciprocal + eviction + output DMA for a closed block."""
                _, _, qw = blk_geom(j)
                ops, sps = blk_state[j]
                rr = small.tile([P, 1], f32, tag="rr", name="rr_t")
                nc.vector.reciprocal(rr[0:qw, :], sps[0:qw, :])
                pieces = 2 if halves_split else 2
                pw = H // pieces
                for pc in range(pieces):
                    outsb = out_pool.tile([P, pw], bf16, tag="outp",
                                          bufs=4, name="outsb_t")
                    if (j + pc) % 2 == 0:
                        nc.scalar.activation(
                            outsb[0:qw, :], ops[0:qw, pc * pw:(pc + 1) * pw],
                            AF.Copy, scale=rr,
                        )
                    else:
                        nc.vector.tensor_scalar_mul(
                            outsb[0:qw, :], ops[0:qw, pc * pw:(pc + 1) * pw],
                            rr,
                        )
                    eng = nc.sync if (j + pc) % 2 == 0 else nc.scalar
                    eng.dma_start(
                        out=out_d[j * P:j * P + qw, pc * pw:(pc + 1) * pw],
                        in_=outsb[0:qw, :],
                    )

            def pv_block(j, i_lo, i_hi, final):
                pv_chains(j, i_lo, i_hi, close=final)
                if final:
                    pv_finish(j, halves_split=False)

            last = NSC - 1
            for i in range(NSC):
                scores_chunk(i)
                if i >= 2:
                    pv_block(i - 2, 0, i - 2, final=True)
            # block 7 full (its eviction overlaps block 8's chains), then
            # block 8 minus its diagonal chunk, then the sums-first
            # continuation so only ~1 matmul + 2 evict/DMA pairs trail.
            pv_block(last - 1, 0, last - 1, final=True)
            pv_block(last, 0, last - 1, final=False)
            gi8, qo8, qw8 = blk_geom(last)
            ops8, sps8 = blk_state[last]
            nc.tensor.matmul(
                sps8[0:qw8, :], lhsT=pts[(last, gi8)][:, qo8:qo8 + qw8],
                rhs=ones_col, start=False, stop=True, skip_group_check=True,
            )
            rr8 = small.tile([P, 1], f32, tag="rr", name="rr_t")
            nc.vector.reciprocal(rr8[0:qw8, :], sps8[0:qw8, :])
            # both half continuations BEFORE any eviction: the tile-granular
            # psum hazard tracking otherwise serializes the second half's
            # matmul behind the first half's eviction.
            for half in range(2):
                nc.tensor.matmul(
                    ops8[0:qw8, half * SB:(half + 1) * SB],
                    lhsT=pts[(last, gi8)][:, qo8:qo8 + qw8],
                    rhs=v[:, last * H + half * SB:last * H + (half + 1) * SB],
                    start=False, stop=True, skip_group_check=True,
                )
            # evict halves on both engines in parallel, then one DMA: a single
            # transfer puts one (final) HBM write receipt on the critical
            # path instead of two.
            outsb8 = out_pool.tile([P, H], bf16, tag="outw",
                                   bufs=1, name="outsb8_t")
            nc.scalar.activation(
                outsb8[0:qw8, 0:SB], ops8[0:qw8, 0:SB],
                AF.Copy, scale=rr8[0:qw8, :],
            )
            nc.vector.tensor_scalar_mul(
                outsb8[0:qw8, SB:H], ops8[0:qw8, SB:H], rr8[0:qw8, :],
            )
            nc.sync.dma_start(
                out=out_d[last * P:last * P + qw8, :],
                in_=outsb8[0:qw8, :],
            )

    nc.compile()
    return nc


def _get_program(q2w=Q2W):
    key = ("nc", q2w)
    if key not in _CACHE:
        _CACHE[key] = _build_program(q2w)
    return _CACHE[key]


def _host_reference(xb, mb, Wq, bq, Wk, bk, Wv, bv):
    """Exact (f64) per-batch fallback, mirrors the reference computation."""
    xb = xb.astype(np.float64)
    Q = xb @ Wq.astype(np.float64).T + bq.astype(np.float64)
    K = xb @ Wk.astype(np.float64).T + bk.astype(np.float64)
    V = xb @ Wv.astype(np.float64).T + bv.astype(np.float64)
    sc = Q @ K.T / np.sqrt(np.float64(H))
    keep = np.tril(np.ones((S, S), bool)) & (mb[None, :] & mb[:, None])
    sc = np.where(keep, sc, -1e9)
    sc -= sc.max(axis=1, keepdims=True)
    Pm = np.exp(sc)
    return ((Pm @ V) / Pm.sum(axis=1, keepdims=True)).astype(np.float32)


def _make_in_maps(x, attention_mask, Wq, bq, Wk, bk, Wv, bv):
    bf16 = ml_dtypes.bfloat16
    f8 = ml_dtypes.float8_e4m3
    f32 = np.float32
    in_maps = []
    fallback = []
    valid_idx = []
    # A = Wq^T Wk  =>  A^T = Wk^T Wq; pack [128p, 8cb, 8h, 128c], fp8 scaled
    AT = (Wk.astype(f32).T @ Wq.astype(f32)) * f32(ASCALE)
    ATp = np.ascontiguousarray(
        AT.reshape(NH, P, NH, P).transpose(1, 2, 0, 3).reshape(P, NH * H)
    ).astype(f8)
    # Wv^T pack half-major [128p, 2half, 8h, 512o]
    WvT = Wv.astype(f32).T.astype(bf16)
    wvp = np.ascontiguousarray(
        WvT.reshape(NH, P, 2, SB).transpose(1, 2, 0, 3).reshape(P, NH * H))
    u = bq.astype(f32) @ Wk.astype(f32)  # [H]; per-k score bias u.x_k
    ii = np.arange(P)
    tri_cap = np.where(
        ii[:, None] > ii[None, :], f32(CAP), f32(3.0e38)
    ).astype(f32)
    for b in range(B):
        mb = attention_mask[b].astype(bool)
        xb = x[b].astype(f32)
        idx = np.nonzero(mb)[0]
        if len(idx) > SC:
            fallback.append(b)
            idx = idx[:SC]
        nk = len(idx)
        valid_idx.append(idx)
        xc = np.zeros((SC, H), dtype=f32)
        xc[:nk] = xb[idx]
        kb = np.full(SC, KBIAS, dtype=f32)
        kb[:nk] = (xc[:nk] @ u) * f32(SCALE)
        smallf = np.concatenate(
            [kb.reshape(NSC, P).T.astype(f32), tri_cap], axis=1)
        # xc^T bf16 packed chunk-major [128p, 9s, 8h, 128c]
        xcT = xc.T.astype(bf16)
        xcp = np.ascontiguousarray(
            xcT.reshape(NH, P, NSC, P).transpose(1, 2, 0, 3).reshape(P, NSC * H))
        # xc^T fp8 (scaled) packed per group, h-major: [128p, 8h, s_in_g, 128c]
        xq8 = (xc.T * f32(XSCALE)).astype(f8)  # [H, SC]
        xq4 = xq8.reshape(NH, P, NSC, P).transpose(1, 0, 2, 3)  # [p, h, s, c]
        def gpack(s0, s1):
            return np.ascontiguousarray(
                xq4[:, :, s0:s1, :].reshape(P, NH * (s1 - s0) * P))
        in_maps.append({
            "ATp": ATp, "wvp": wvp,
            "xcp": np.ascontiguousarray(xcp),
            "xq0": gpack(0, 4), "xq1": gpack(4, 8), "xq2": gpack(8, 9),
            "smallf": np.ascontiguousarray(smallf),
        })
    return in_maps, fallback, valid_idx


def run_spmd(x, attention_mask, Wq, bq, Wk, bk, Wv, bv, **spmd_kwargs):
    """Build (cached), run on 8 cores, return (stacked output, BassKernelResults)."""
    from concourse import bass_utils

    counts = [int(attention_mask[b].astype(bool).sum()) for b in range(B)]
    q2w = min(P, max(1, max(counts) - 8 * P))
    nc = _get_program(q2w)
    in_maps, fallback, valid_idx = _make_in_maps(
        x, attention_mask, Wq, bq, Wk, bk, Wv, bv)
    res = bass_utils.run_bass_kernel_spmd(
        nc, in_maps, core_ids=list(range(B)), **spmd_kwargs
    )
    bvf = bv.astype(np.float32)
    out = np.empty((B, S, H), dtype=np.float32)
    for b in range(B):
        dev = np.asarray(res.results[b]["out"], dtype=np.float32)
        idx = valid_idx[b]
        # scatter compacted rows back; bv was dropped from the device V
        # projection and attn rows sum to 1, so += bv here is exact.
        out[b][idx] = dev[:len(idx)] + bvf
        inv = ~attention_mask[b].astype(bool)
        if inv.any():
            # fully-padded query rows reduce to the uniform mean of all V
            # rows; mean(V) == mean(x) @ Wv.T + bv by linearity.
            mv = (x[b].astype(np.float64).mean(axis=0) @
                  Wv.astype(np.float64).T + bv.astype(np.float64))
            out[b][inv] = mv.astype(np.float32)
    for b in fallback:  # mask had > SC valid keys (never with ~50% masks)
        out[b] = _host_reference(x[b], attention_mask[b].astype(bool),
                                 Wq, bq, Wk, bk, Wv, bv)
    return out, res


def kernel(x, attention_mask, Wq, bq, Wk, bk, Wv, bv):
    x = np.asarray(x)
    attention_mask = np.asarray(attention_mask)
    Wq, bq = np.asarray(Wq), np.asarray(bq)
    Wk, bk = np.asarray(Wk), np.asarray(bk)
    Wv, bv = np.asarray(Wv), np.asarray(bv)
    out, _ = run_spmd(x, attention_mask, Wq, bq, Wk, bk, Wv, bv)
    return out
